# revision 8
# baseline (speedup 1.0000x reference)
"""SAGAN-style self-attention block on 8 TRN2 NeuronCores.

Data-parallel over batch (B=8): core i processes sample i with replicated
(tiny) conv weights. No collectives.

Per-core math (pix = 64*64 = 4096, C=256, M = 32*32 = 1024 pooled keys):
  g = x @ Wg                      [4096, 32]
  f = maxpool2x2(x @ Wf)          [1024, 32]
  h = maxpool2x2(x @ Wh)          [1024, 128]
  s = g @ f.T                     [4096, 1024]
  beta = softmax(s, -1)
  o = beta @ h                    [4096, 128]
  out = gamma * (o @ Wo) + x      [4096, 256]

v3 design:
  - x loaded ONLY as bf16 [256, 4096] (2MB); residual + output are bf16.
  - g and f convs share ONE col-packed wave (col groups 0/1); the 4-band
    replicas needed for 4-row-packed K=32 s-matmuls are made by two
    partition-doubling SBUF->SBUF DMAs per tensor per chunk.
  - h transposed to [m, c'] by DMA-transpose (xbar), not TensorE.
  - softmax denominator: 2 col-packed waves of M=32 all-ones matmuls
    + one K=128 all-ones matmul that sums bands and broadcasts 32*r;
    gamma*32 folded into Wo on the host.
  - exp split: mi 0-5 on ACT (real exp), mi 6-7 on DVE via the Schraudolph
    bit trick (uint16((s*A+B)/2^16) IS bf16(exp(s)) to ~3%), which costs the
    same as the mandatory PSUM->SBUF drain.
"""

import numpy as np

import concourse.bass as bass
import concourse.mybir as mybir
from concourse import bacc
import concourse.tile as tile
from concourse.bass_utils import run_bass_kernel_spmd
from concourse.masks import make_identity

F32 = mybir.dt.float32
BF16 = mybir.dt.bfloat16
U16 = mybir.dt.uint16

P = 128
NPIX = 4096          # 64*64 pixels
NCHUNK = 8           # pixel chunks of 512
PIX = NPIX // NCHUNK  # 512
M = 1024             # pooled keys
MCH = 8              # m chunks of 128
C = 256              # channels (2 k-chunks of 128)
C8 = 32              # C//8
C2 = 128             # C//2

# Schraudolph: f32 bits of exp(x) ~ A*x + B; uint16 bf16-bits = that / 2^16
SCH_A = float((2 ** 23) / np.log(2) / 65536.0)
SCH_B = float((127 * (2 ** 23) - 366393.0) / 65536.0)

_CACHED = {}


def _build():
    nc = bacc.Bacc()

    xb_ext = nc.declare_dram_parameter("xb", [C, NPIX], BF16, isOutput=False)
    wf_ext = nc.declare_dram_parameter("Wf", [C, C8], F32, isOutput=False)
    wg_ext = nc.declare_dram_parameter("Wg", [C, C8], F32, isOutput=False)
    wh_ext = nc.declare_dram_parameter("Wh", [C, C2], F32, isOutput=False)
    wo_ext = nc.declare_dram_parameter("Wo", [C2, C], F32, isOutput=False)
    out_ext = nc.declare_dram_parameter("out", [C, NPIX], BF16, isOutput=True)

    xb_r = xb_ext.rearrange("(j p) n -> p j n", p=P)
    out_r = out_ext.rearrange("(j p) n -> p j n", p=P)

    with tile.TileContext(nc) as tc:
        with (
            tc.tile_pool(name="const", bufs=1) as constp,
            tc.tile_pool(name="big", bufs=1) as bigp,
            tc.tile_pool(name="outp", bufs=3) as outp,
            tc.tile_pool(name="ps1", bufs=4, space="PSUM") as ps1,
            tc.tile_pool(name="pss", bufs=2, space="PSUM") as pss,
        ):
            # ---- PE warm-up first: only dep is one DVE memset ------------
            dummy = constp.tile([P, PIX], BF16)
            nc.vector.memset(dummy, 0.0)
            pw = ps1.tile([P, PIX], F32, tag="pb")
            for w in range(10):
                nc.tensor.matmul(pw, lhsT=dummy[:, 0:P], rhs=dummy, start=(w == 0),
                                 stop=(w == 9))

            # ---- big persistent activations ------------------------------
            xb_sb = bigp.tile([P, 2, NPIX], BF16)         # bf16 x (convs + resid)
            gt_sb = bigp.tile([P, NPIX], BF16)            # g^T, 4 replicated bands
            ft_sb = bigp.tile([P, M], BF16)               # f^T pooled, 4 bands
            ht_sb = bigp.tile([C2, M], BF16)              # h^T pooled [c', m]
            h_sb = bigp.tile([P, MCH, C2], BF16)          # h [m, c'] per m-chunk
            et_sb = bigp.tile([P, MCH, NPIX], U16)        # e^T as bf16 bits
            ot_sb = bigp.tile([C2, NPIX], BF16)           # o^T scaled
            rsum_sb = bigp.tile([P, NPIX], BF16)          # band-partial denom sums
            scale_sb = bigp.tile([P, NPIX], F32)          # 1/(32 r) replicated

            def ns(n):
                return slice(n * PIX, (n + 1) * PIX)

            def et_bf(mi, n):
                return et_sb[:, mi, ns(n)].bitcast(BF16)

            # ---- input DMAs first so chunk 0 lands ASAP ------------------
            for n in range(NCHUNK):
                nc.sync.dma_start(out=xb_sb[:, :, ns(n)], in_=xb_r[:, :, ns(n)])

            # ---- constants / weights -------------------------------------
            wg_f32 = constp.tile([P, 2, C8], F32)
            nc.sync.dma_start(out=wg_f32, in_=wg_ext.rearrange("(ko p) m -> p ko m", p=P))
            wf_f32 = constp.tile([P, 2, C8], F32)
            nc.sync.dma_start(out=wf_f32, in_=wf_ext.rearrange("(ko p) m -> p ko m", p=P))
            wh_f32 = constp.tile([P, 2, C2], F32)
            nc.sync.dma_start(out=wh_f32, in_=wh_ext.rearrange("(ko p) m -> p ko m", p=P))
            wo_f32 = constp.tile([C2, C], F32)
            nc.sync.dma_start(out=wo_f32, in_=wo_ext[:])

            wg_bf = constp.tile([P, 2, C8], BF16)
            nc.vector.tensor_copy(out=wg_bf, in_=wg_f32)
            wf_bf = constp.tile([P, 2, C8], BF16)
            nc.vector.tensor_copy(out=wf_bf, in_=wf_f32)
            wh_bf = constp.tile([P, 2, C2], BF16)
            nc.vector.tensor_copy(out=wh_bf, in_=wh_f32)
            wo_bf = constp.tile([C2, 2, P], BF16)
            nc.vector.tensor_copy(out=wo_bf, in_=wo_f32.rearrange("k (j m) -> k j m", j=2))

            ones32 = constp.tile([P, C8], BF16)
            nc.vector.memset(ones32, 1.0)
            ones128 = constp.tile([P, P], BF16)
            nc.vector.memset(ones128, 1.0)

            # ---- phase A: 4-band g/f convs, h; pools ----------------------
            for n in range(NCHUNK):
                xr = xb_sb[:, :, ns(n)]
                pgf = pss.tile([P, 2, PIX], F32, tag="s")
                pg = pgf[:, 0]
                pf = pgf[:, 1]
                ph = ps1.tile([P, PIX], F32, tag="pb")
                for ko in range(2):
                    for cg in range(4):
                        nc.tensor.matmul(pg[cg * 32:(cg + 1) * 32],
                                         lhsT=wg_bf[:, ko], rhs=xr[:, ko],
                                         start=(ko == 0), stop=(ko == 1),
                                         tile_position=(0, cg * 32))
                    for cg in range(4):
                        nc.tensor.matmul(pf[cg * 32:(cg + 1) * 32],
                                         lhsT=wf_bf[:, ko], rhs=xr[:, ko],
                                         start=(ko == 0), stop=(ko == 1),
                                         tile_position=(0, cg * 32))
                    nc.tensor.matmul(ph, lhsT=wh_bf[:, ko], rhs=xr[:, ko],
                                     start=(ko == 0), stop=(ko == 1))
                mc = slice(n * 128, (n + 1) * 128)
                nc.scalar.copy(out=gt_sb[:, ns(n)], in_=pg)
                pfv = pf.rearrange("p (r a c b) -> p r c a b", r=4, a=2, b=2)
                nc.vector.tensor_reduce(out=ft_sb[:, mc], in_=pfv,
                                        axis=mybir.AxisListType.XY, op=mybir.AluOpType.max)
                phv = ph.rearrange("p (r a c b) -> p r c a b", r=4, a=2, b=2)
                nc.vector.tensor_reduce(out=ht_sb[:, mc], in_=phv,
                                        axis=mybir.AxisListType.XY, op=mybir.AluOpType.max)
                # h chunk -> [m, c'] via xbar DMA transpose
                nc.sync.dma_start_transpose(out=h_sb[:, n], in_=ht_sb[:, mc])

            # ---- phase B -------------------------------------------------
            def emit_s_exp(n):
                # two 4-row-packed K=32 waves; exp: ACT for mi 0-5,
                # Schraudolph on DVE for mi 6-7
                for half in range(2):
                    ps_t = pss.tile([P, 2, PIX], F32, tag="s")
                    ps_t2 = pss.tile([P, 2, PIX], F32, tag="s")
                    for q in range(4):
                        mi = 4 * half + q
                        dst = ps_t if q < 2 else ps_t2
                        nc.tensor.matmul(dst[:, q % 2],
                                         lhsT=ft_sb[32 * q:32 * (q + 1),
                                                    mi * P:(mi + 1) * P],
                                         rhs=gt_sb[32 * q:32 * (q + 1), ns(n)],
                                         start=True, stop=True,
                                         tile_position=(32 * q, 0))
                    base = 4 * half
                    nc.scalar.activation(
                        out=et_sb[:, base:base + 2, ns(n)].bitcast(BF16),
                        in_=ps_t, func=mybir.ActivationFunctionType.Exp)
                    if half == 0:
                        nc.scalar.activation(
                            out=et_sb[:, base + 2:base + 4, ns(n)].bitcast(BF16),
                            in_=ps_t2, func=mybir.ActivationFunctionType.Exp)
                    else:
                        for q in range(2):
                            nc.vector.tensor_scalar(
                                out=et_sb[:, base + 2 + q, ns(n)],
                                in0=ps_t2[:, q], scalar1=SCH_A, scalar2=SCH_B,
                                op0=mybir.AluOpType.mult, op1=mybir.AluOpType.add)

            def emit_post(n):
                # denominator: 2 col-packed waves of M=32 ones-matmuls
                pr = ps1.tile([P, PIX], F32, tag="pb")
                for rnd in range(2):
                    for cg in range(4):
                        mi = 4 * rnd + cg
                        nc.tensor.matmul(pr[cg * 32:(cg + 1) * 32], lhsT=ones32,
                                         rhs=et_bf(mi, n),
                                         start=(rnd == 0), stop=(rnd == 1),
                                         tile_position=(0, cg * 32))
                nc.scalar.copy(out=rsum_sb[:, ns(n)], in_=pr)
                # sum the 4 bands AND broadcast to 128 partitions: 32*r
                pb = ps1.tile([P, PIX], F32, tag="pb")
                nc.tensor.matmul(pb, lhsT=ones128, rhs=rsum_sb[:, ns(n)],
                                 start=True, stop=True)
                nc.vector.reciprocal_approx_fast(out=scale_sb[:, ns(n)], in_=pb)
                # o = e @ h (unnormalized), then scale by 1/(32 r)
                po = ps1.tile([P, PIX], F32, tag="pb")
                for mi in range(MCH):
                    nc.tensor.matmul(po, lhsT=h_sb[:, mi], rhs=et_bf(mi, n),
                                     start=(mi == 0), stop=(mi == MCH - 1))
                nc.vector.tensor_tensor(out=ot_sb[:, ns(n)], in0=po,
                                        in1=scale_sb[:, ns(n)], op=mybir.AluOpType.mult)
                # final conv (gamma*32 folded into Wo) + residual, bf16 out
                for j in range(2):
                    pf2 = ps1.tile([P, PIX], F32, tag="pb")
                    nc.tensor.matmul(pf2, lhsT=wo_bf[:, j], rhs=ot_sb[:, ns(n)],
                                     start=True, stop=True)
                    ob = outp.tile([P, PIX], BF16)
                    nc.vector.tensor_tensor(out=ob, in0=pf2, in1=xb_sb[:, j, ns(n)],
                                            op=mybir.AluOpType.add)
                    nc.sync.dma_start(out=out_r[:, j, ns(n)], in_=ob)

            # software-pipelined: exp producer one chunk ahead of consumers
            emit_s_exp(0)
            for n in range(1, NCHUNK):
                emit_s_exp(n)
                emit_post(n - 1)
            emit_post(NCHUNK - 1)

    nc.finalize()
    return nc


def _get_nc():
    if "nc" not in _CACHED:
        _CACHED["nc"] = _build()
    return _CACHED["nc"]


def _make_in_maps(inputs):
    x = np.asarray(inputs["x"], dtype=np.float32)        # [8, 64, 64, 256]
    B = x.shape[0]
    for bname in ("bf", "bg", "bh", "bo"):
        b = np.asarray(inputs[bname])
        assert np.max(np.abs(b)) == 0.0, f"{bname} must be zero (spec fill=zeros)"
    gamma = float(np.asarray(inputs["gamma"]).reshape(-1)[0])
    wo_eff = (np.asarray(inputs["Wo"], dtype=np.float32) * (gamma * 32.0)
              ).astype(np.float32)
    wf = np.ascontiguousarray(np.asarray(inputs["Wf"], dtype=np.float32))
    wg = np.ascontiguousarray(np.asarray(inputs["Wg"], dtype=np.float32))
    wh = np.ascontiguousarray(np.asarray(inputs["Wh"], dtype=np.float32))
    import ml_dtypes
    in_maps = []
    for i in range(B):
        xb = np.ascontiguousarray(x[i].reshape(NPIX, C).T).astype(ml_dtypes.bfloat16)
        in_maps.append({"xb": xb, "Wf": wf, "Wg": wg, "Wh": wh, "Wo": wo_eff})
    return in_maps


def _gather(results):
    outs = []
    for r in results:
        ot = np.asarray(r["out"]).astype(np.float32)   # [256, 4096]
        outs.append(ot.T.reshape(64, 64, C))
    return np.stack(outs).astype(np.float32)


def kernel(**inputs):
    nc = _get_nc()
    in_maps = _make_in_maps(inputs)
    res = run_bass_kernel_spmd(nc, in_maps, core_ids=list(range(len(in_maps))))
    return _gather(res.results)


def bench(inputs, trace=True):
    """Run with profiling; returns (output, BassKernelResults)."""
    nc = _get_nc()
    in_maps = _make_in_maps(inputs)
    res = run_bass_kernel_spmd(nc, in_maps, core_ids=list(range(len(in_maps))),
                               trace=trace)
    return _gather(res.results), res
